# revision 89
# baseline (speedup 1.0000x reference)
"""Trainium2 Bass kernel for the dual-stream "DifAttention" block — v16.

Partitioning: 8 independent (batch, stream) units, one per core, SPMD, no
collectives:
    x-core b: t_qk=x[b], t_v=x[b], t_qo=y[b]
    y-core b: t_qk=y[b], t_v=x[b], t_qo=x[b]

Design (measured-model driven; ~277us median, rel err ~1.1e-2):

  projections      plain bf16, 12 matmuls per output col-tile (6144 cyc/co).
                   Measured: the v3 split-fp8 3-term DoubleRow form costs
                   9216 cyc/co — DR halves per-term cycles but the 3 terms
                   cost 1.5x bf16. 1-term fp8 DR is no faster either: the
                   512-col matmul pace (~250ns) is dispatch-bound, not ALU-
                   bound. bf16 is fastest AND most accurate here.
  S^T = K Q^T      bf16 blockdiag: stationary [128,128] = blockdiag(k[d,mA],
                   k[d,mB]), moving = q duplicated across partition halves.
                   512 cyc per [128m x 512n]; only bf16 cast error survives.
  exp split        ACT does 6 of 8 m-tiles per (head,att) via native EXP;
                   DVE does 2 of 8 via a Schraudolph bit-trick:
                   a = bitcast_bf16(int16(s*A + B)), A = +-0.125*log2e*128,
                   B = 128*(127 + c), c = -0.0564 (mean-zero centering: no
                   softmax-mass bias between DVE and ACT tiles). This keeps
                   the exp stream off the critical path (PE is the pacer).
  A V              o[n,d] form: stationary = A^T tile [128m x 128n] bf16,
                   moving = V[m, 64d + ones-col]; the softmax denominator
                   lands as a per-partition column. ~59ns/matmul issue rate
                   (dispatch floor — fp8 stationary does NOT help).
  out proj         bf16 from onorm^T (PE-transposed via identity matmuls).

Scheduling: per-c-tile input DMAs (contiguous source rows, ~4x faster than
a whole-tensor rearrange walk), explicit (head idx, weave mt) filler
schedule with deferred sinks (mm@2/sink@7) so projection casts are ready
when the in-order DVE queue reaches them; separate PSUM pools for s-tiles
(2 bufs), o (1), fillers/transposes/out-proj (1) = 8 banks exactly. The
prologue projections rotate through the (still idle) s/o psum slots so
they don't serialize behind each other's DVE casts; the output is stored
bf16 (tail is store-drain bound; b_proj added host-side in fp32).
"""

import numpy as np
import ml_dtypes

import concourse.bass as bass
import concourse.bacc as bacc
import concourse.tile as tile
from concourse import mybir
from concourse.bass_utils import run_bass_kernel_spmd

P = 128
B, N, C = 4, 1024, 768
H, HD = 12, 64
CT = C // P           # 6 column tiles (= head pairs)
NT = N // P           # 8 sequence tiles
EXPSC = 0.125         # 1/sqrt(hd)

LOG2E = 1.4426950408889634
C_CENTER = -0.056401  # mean-zero Schraudolph centering
EXP_A = EXPSC * LOG2E * 128.0
EXP_B = 128.0 * (127.0 + C_CENTER)
DVE_MTS = (2, 5)      # m-tiles per (head,att) handled by the DVE exp
# (1,4,6) with a third DVE tile measured time-equivalent within run
# variance but +0.7e-3 err — the depth-2 s-ring micro-stalls (~14us of
# sub-400ns gaps in the filler-light heads) are not drain-rate-bound.

FP32 = mybir.dt.float32
BF16 = mybir.dt.bfloat16
FP8 = mybir.dt.float8e4
I16 = mybir.dt.int16
EXP = mybir.ActivationFunctionType.Exp
MUL = mybir.AluOpType.mult
ADD = mybir.AluOpType.add


def build_kernel():
    nc = bacc.Bacc("TRN2", target_bir_lowering=False, debug=False,
                   num_devices=8)

    d_qk = nc.dram_tensor("qk16", [C, N], BF16, kind="ExternalInput")
    d_qo = nc.dram_tensor("qo16", [C, N], BF16, kind="ExternalInput")
    d_v = nc.dram_tensor("v16", [C, N], BF16, kind="ExternalInput")
    d_wq = nc.dram_tensor("wq16", [C, C], BF16, kind="ExternalInput")
    d_wk = nc.dram_tensor("wk16", [C, C], BF16, kind="ExternalInput")
    d_wqo = nc.dram_tensor("wqo16", [C, C], BF16, kind="ExternalInput")
    d_wv = nc.dram_tensor("wv16", [C, C], BF16, kind="ExternalInput")
    d_wp = nc.dram_tensor("wp16", [C, C], BF16, kind="ExternalInput")
    d_eye = nc.dram_tensor("eye16", [P, P], BF16, kind="ExternalInput")
    # bf16 output: halves the tail store traffic (the kernel end is
    # store-drain bound); b_proj is added host-side in fp32
    d_out = nc.dram_tensor("out", [N, C], BF16, kind="ExternalOutput")

    with tile.TileContext(nc) as tc:
        _body(tc, d_qk, d_qo, d_v, d_wq, d_wk, d_wqo, d_wv, d_wp, d_eye,
              d_out)
    nc.compile()
    return nc


def _body(tc, d_qk, d_qo, d_v, d_wq, d_wk, d_wqo, d_wv, d_wp, d_eye, d_out):
    nc = tc.nc
    _ap = lambda t: t if isinstance(t, bass.AP) else t.ap()
    d_qk, d_qo, d_v, d_wq, d_wk, d_wqo, d_wv, d_wp, d_eye, d_out = (
        _ap(t) for t in (d_qk, d_qo, d_v, d_wq, d_wk, d_wqo, d_wv, d_wp,
                         d_eye, d_out))
    from contextlib import ExitStack
    ctx = ExitStack()
    xpool = ctx.enter_context(tc.tile_pool(name="xpool", bufs=2))
    wpool = ctx.enter_context(tc.tile_pool(name="wpool", bufs=2))
    spool = ctx.enter_context(tc.tile_pool(name="spool", bufs=2))
    persist = ctx.enter_context(tc.tile_pool(name="persist", bufs=1))
    apool = ctx.enter_context(tc.tile_pool(name="apool", bufs=2))
    rpool = ctx.enter_context(tc.tile_pool(name="rpool", bufs=2))
    tpool = ctx.enter_context(tc.tile_pool(name="tpool", bufs=2))
    opool = ctx.enter_context(tc.tile_pool(name="opool", bufs=2))
    psS = ctx.enter_context(tc.tile_pool(name="psS", bufs=2, space="PSUM"))
    psO = ctx.enter_context(tc.tile_pool(name="psO", bufs=1, space="PSUM"))
    psP = ctx.enter_context(tc.tile_pool(name="psP", bufs=1, space="PSUM"))

    # ---- persistent tensors -------------------------------------------------
    # kblk[k, co, h, mt, M]: bf16 blockdiag stationary per (co,h,mt):
    #   rows 0-63 x cols 0-63 = k[d, mA], rows 64-127 x cols 64-127 = k[d, mB]
    # (fp8 kblk was time-neutral — S pacing is dispatch-bound — so bf16 buys
    # back ~3e-3 of error for free, funding the bf16 output store)
    kblk = persist.tile([P, CT, 2, NT, P], BF16, name="kblk")
    # q duplicated to both partition halves; per-co overwritten by qo after
    # the (co, att=0, *) heads have been emitted (emission order = dep order)
    qdup = persist.tile([P, CT, 2, N], BF16, name="qdup")
    # V[m, head, d] bf16 with a ones column at d=64 (softmax denominator)
    vsb = persist.tile([P, NT, H, HD + 1], BF16, name="vsb")
    onorm = persist.tile([P, NT, C], BF16, name="onorm")   # O[n, c]
    onormT = persist.tile([P, CT, N], BF16, name="onormT")  # O^T[c, n]
    eye = persist.tile([P, P], BF16, name="eye")

    # zero the off-diagonal blocks of kblk on DVE; one memset per
    # (co, partition-half) covers both h (contiguous nt-rows)
    def zmemset(co):
        for parts, coff in ((slice(0, 64), 64), (slice(64, P), 0)):
            base = kblk[parts, co, 0, 0, coff:coff + 64]
            dst = bass.AP(tensor=base.tensor, offset=base.offset,
                          ap=[list(base.ap[0]), [P, 2 * NT], [1, 64]])
            nc.vector.memset(dst, 0.0)

    # ---- load inputs --------------------------------------------------------
    # Per-c-tile DMAs: each reads 128 CONTIGUOUS source rows (the whole-tensor
    # rearrange walks the source p-major = 768 scattered row reads ~79GB/s;
    # split loads measured ~4x faster)
    def load_split(dst, d_src, eng):
        for t in range(CT):
            eng.dma_start(dst[:, t, :], d_src[t * P:(t + 1) * P, :])

    xqk = xpool.tile([P, CT, N], BF16, tag="x16", name="xqk")
    xqo = xpool.tile([P, CT, N], BF16, tag="x16", name="xqo")
    xv = xpool.tile([P, CT, N], BF16, tag="x16b", name="xv", bufs=1)
    load_split(xqk, d_qk, nc.sync)
    nc.sync.dma_start(eye[:], d_eye)
    # xv split across two rings so it lands ~4us earlier for the vmt fillers
    for t in range(3):
        nc.sync.dma_start(xv[:, t, :], d_v[t * P:(t + 1) * P, :])

    # ---- phase 1: projections (bf16), co-granular. The matmul part and the
    # sink (cast + scatter) are scheduled separately: the sink's DVE cast
    # sits in the same in-order queue as the exp stream, so it must only be
    # reached once its psum is long finished ---------------------------------
    def make_proj(d_w, srcx, name, eng=None):
        wsb = wpool.tile([P, CT, C], BF16, tag="w16", name=name, bufs=3)
        load_split(wsb, d_w, eng or nc.gpsimd)

        def mm_fn(co, ps=None):
            if ps is None:
                ps = psP.tile([P, N], FP32, tag="p", name="ps_qkv")
            cosl = slice(co * P, (co + 1) * P)
            for ch in range(2):
                nsl = slice(ch * 512, (ch + 1) * 512)
                for j in range(CT):
                    nc.tensor.matmul(
                        ps[:, nsl], wsb[:, j, cosl], srcx[:, j, nsl],
                        start=(j == 0), stop=(j == CT - 1))
            return ps
        return mm_fn

    def dup_sink(co, ps):
        stg = spool.tile([P, N], BF16, tag="stg", name="qstg")
        nc.vector.tensor_copy(stg[:], ps[:])
        nc.gpsimd.dma_start(qdup[0:64, co, 0, :], stg[0:64, :])
        nc.gpsimd.dma_start(qdup[64:P, co, 0, :], stg[0:64, :])
        nc.gpsimd.dma_start(qdup[0:64, co, 1, :], stg[64:P, :])
        nc.gpsimd.dma_start(qdup[64:P, co, 1, :], stg[64:P, :])

    # ramp variant: cast + scatter per 512-col half, so the first S matmuls
    # can start as soon as the first half of the co=0 projection lands
    def dup_sink_split(co, ps):
        stg = spool.tile([P, N], BF16, tag="stg", name="qstg")
        for hf in range(2):
            nsl = slice(hf * 512, (hf + 1) * 512)
            nc.vector.tensor_copy(stg[:, nsl], ps[:, nsl])
            nc.gpsimd.dma_start(qdup[0:64, co, 0, nsl], stg[0:64, nsl])
            nc.gpsimd.dma_start(qdup[64:P, co, 0, nsl], stg[0:64, nsl])
            nc.gpsimd.dma_start(qdup[0:64, co, 1, nsl], stg[64:P, nsl])
            nc.gpsimd.dma_start(qdup[64:P, co, 1, nsl], stg[64:P, nsl])

    def _k_scatter(co, stg, mts):
        def stg_ap(prt, half):
            s = stg[prt, mts.start * P + half * 64:
                    mts.start * P + half * 64 + 64]
            return bass.AP(tensor=s.tensor, offset=s.offset,
                           ap=[list(s.ap[0]), [P, mts.stop - mts.start],
                               [1, 64]])
        nc.gpsimd.dma_start(kblk[0:64, co, 0, mts, 0:64],
                            stg_ap(slice(0, 64), 0))
        nc.gpsimd.dma_start(kblk[64:P, co, 0, mts, 64:P],
                            stg_ap(slice(0, 64), 1))
        nc.gpsimd.dma_start(kblk[0:64, co, 1, mts, 0:64],
                            stg_ap(slice(64, P), 0))
        nc.gpsimd.dma_start(kblk[64:P, co, 1, mts, 64:P],
                            stg_ap(slice(64, P), 1))

    # k stages through fp8: halves the scatter bytes on the gpsimd ring
    # (which also carries the q dups that gate the S matmuls); the k noise
    # (~1.8% rms) costs ~3e-3 output err — measured worth the ~3us
    def k_sink(co, ps):
        stg = spool.tile([P, N], FP8, tag="stg8", name="kstg")
        nc.vector.tensor_copy(stg[:], ps[:])
        _k_scatter(co, stg, slice(0, NT))

    def k_sink_split(co, ps):
        stg = spool.tile([P, N], FP8, tag="stg8", name="kstg")
        for hf in range(2):
            nsl = slice(hf * 512, (hf + 1) * 512)
            nc.vector.tensor_copy(stg[:, nsl], ps[:, nsl])
            _k_scatter(co, stg, slice(hf * 4, hf * 4 + 4))

    def make_vproj():
        wsb = wpool.tile([P, CT, C], BF16, tag="w16", name="wv16", bufs=3)
        load_split(wsb, d_wv, nc.gpsimd)

        def mt_fn(mt, ps=None):
            if ps is None:
                ps = psP.tile([P, N], FP32, tag="p", name="ps_v")
            msl = slice(mt * P, (mt + 1) * P)
            for base, wd in ((0, 512), (512, 256)):
                for j in range(CT):
                    nc.tensor.matmul(
                        ps[:, base:base + wd],
                        xv[:, j, msl], wsb[:, j, base:base + wd],
                        start=(j == 0), stop=(j == CT - 1))
            nc.vector.tensor_copy(
                vsb[:, mt, :, 0:HD],
                ps[:, 0:C].rearrange("p (h d) -> p h d", h=H))
        return mt_fn

    # ---- phase 2: attention, pipelined at (p, att, head) granularity --------
    # AV slices of the previous head are woven uniformly (1 per weave step,
    # 2 at step 6) so each step carries ~1us of PE work to match the exp
    # drain cadence; norm at step 6 still frees o a step before the next
    # head's o allocation needs it (psO has 1 buf).
    AV_SCHED = {0: (0,), 1: (1,), 2: (2,), 3: (3,), 4: (4,), 5: (5,),
                6: (6, 7)}

    def emit_av_slice(pend, nt):
        pp, patt, ph, a, o = pend
        hh = 2 * pp + ph
        ntsl = slice(nt * P, (nt + 1) * P)
        for mt in range(NT):
            nc.tensor.matmul(
                o[:, nt, 0:HD + 1], a[:, mt, ntsl],
                vsb[:, mt, hh, :],
                start=(mt == 0), stop=(mt == NT - 1),
                skip_group_check=True)

    def emit_norm(pend):
        pp, patt, ph, a, o = pend
        hh = 2 * pp + ph
        r = rpool.tile([P, NT, 1], FP32, tag="r", name="r_den")
        nc.vector.reciprocal(r[:], o[:, :, HD:HD + 1])
        rb = bass.AP(tensor=r.tensor, offset=r[:].offset,
                     ap=[list(r[:].ap[0]), [1, NT], [0, HD]])
        dst = onorm[:, :, hh * HD:(hh + 1) * HD]
        if patt == 0:
            nc.vector.tensor_mul(dst, o[:, :, 0:HD], rb)
        else:
            t = tpool.tile([P, NT, HD], BF16, tag="t", name="t_norm")
            nc.vector.tensor_mul(t[:], o[:, :, 0:HD], rb)
            nc.vector.tensor_add(dst, dst, t[:])

    def emit_transpose(p):
        trp = psP.tile([P, N], BF16, tag="p", name="tr")
        for nt in range(NT):
            nc.tensor.transpose(trp[:, nt * P:(nt + 1) * P],
                                onorm[:, nt, p * P:(p + 1) * P], eye[:])
        nc.vector.tensor_copy(onormT[:, p, :], trp[:])

    def emit_head(idx, p, att, h, pend, sched):
        sgn = 1.0 if att == 0 else -1.0
        a = apool.tile([P, NT, N], BF16, tag="a", name="a_att")
        if pend is not None:
            o = psO.tile([P, NT, P], FP32, tag="o", name="o_av")
            pend = pend + (o,)
        for mt in range(NT):
            # in the filler-light back-half heads the psP slot is idle;
            # borrowing it for one s-tile per head gives the depth-2 s-ring
            # a mid-unit relief point (PE can run one step further ahead of
            # the exp drain, cutting the per-step lockstep stalls)
            if (mt == 4 and idx in (12, 14, 15, 16, 18, 19, 21, 22, 23)) \
                    or (mt == 6 and idx in (12, 15, 16, 19, 21, 23)):
                s = psP.tile([P, N], FP32, tag="p", name="s_extra")
            else:
                s = psS.tile([P, N], FP32, tag="s", name="s_att")
            for ch in range(2):
                nsl = slice(ch * 512, (ch + 1) * 512)
                nc.tensor.matmul(
                    s[:, nsl], kblk[:, p, h, mt, :], qdup[:, p, h, nsl],
                    start=True, stop=True)
            if mt in DVE_MTS:
                nc.vector.tensor_scalar(
                    a[:, mt, :].bitcast(I16), s[:],
                    sgn * EXP_A, EXP_B, MUL, ADD)
            else:
                nc.scalar.activation(a[:, mt, :], s[:], EXP,
                                     scale=sgn * EXPSC)
            if pend is not None:
                for nt in AV_SCHED.get(mt, ()):
                    emit_av_slice(pend, nt)
                if mt == 6:
                    emit_norm(pend)
                    if pend[1] == 1 and pend[2] == 1:
                        emit_transpose(pend[0])
            for fn in sched.get((idx, mt), ()):
                fn()
        return a

    # weight loads: wq then xqo on the scalar queue, wqo on sync (its
    # w16-ring slot frees only after the vmt fillers, so its DMA must not
    # block the gpsimd scatters or the first exps), the rest on gpsimd
    kmm = make_proj(d_wk, xqk, "wk16")
    qmm = make_proj(d_wq, xqk, "wq16", eng=nc.scalar)
    for t in range(3, CT):
        nc.scalar.dma_start(xv[:, t, :], d_v[t * P:(t + 1) * P, :])
    load_split(xqo, d_qo, nc.scalar)
    vmt = make_vproj()
    qomm = make_proj(d_wqo, xqo, "wqo16", eng=nc.sync)

    # prologue: zeros + co 0 and 4 of Q/K inline (the PE would otherwise
    # stall on input DMA here anyway), vsb tiles 0-3 before the first AV.
    # The attention-phase psum slots (psS "s", psO "o") are still free here,
    # so rotate the prologue projections through them — a single psP slot
    # would serialize each co_fn behind the previous one's DVE cast.
    def pro_ps():
        i = 0
        while True:
            yield psP.tile([P, N], FP32, tag="p", name="ps_pro")
            yield psS.tile([P, N], FP32, tag="s", name="ps_pro2")
            yield psO.tile([P, NT, P], FP32, tag="o",
                           name="ps_pro3").rearrange("p a b -> p (a b)")
            i += 1
    pro = pro_ps()
    for co in range(CT):
        zmemset(co)
    # q first (wq's ring is shorter, so it lands before wk finishes), and
    # half-granular sinks so the first S matmuls start ~3us earlier
    dup_sink_split(0, qmm(0, next(pro)))
    k_sink_split(0, kmm(0, next(pro)))
    nc.vector.memset(vsb[:, :, :, HD:HD + 1], 1.0)
    # co=4 before the vmt tiles: the vmt matmuls wait on the xv DMA anyway,
    # and this keeps the PE busy through that window
    k_sink(4, kmm(4, next(pro)))
    dup_sink(4, qmm(4, next(pro)))
    wp = wpool.tile([P, CT, C], BF16, tag="wf", name="wp", bufs=1)
    load_split(wp, d_wp, nc.gpsimd)

    heads = [(0, 0, 0), (0, 0, 1), (4, 0, 0), (4, 0, 1), (1, 0, 0),
             (1, 0, 1), (0, 1, 0), (0, 1, 1), (4, 1, 0), (4, 1, 1),
             (2, 0, 0), (2, 0, 1), (1, 1, 0), (1, 1, 1), (3, 0, 0),
             (3, 0, 1), (2, 1, 0), (2, 1, 1), (5, 0, 0), (5, 0, 1),
             (3, 1, 0), (3, 1, 1), (5, 1, 0), (5, 1, 1)]
    # Explicit filler schedule, (head idx, weave mt) -> work. Each projection
    # emits its matmuls early in a head (mt=2) and its sink (DVE cast +
    # gpsimd scatter) late (mt=7), so the cast is ready by the time the DVE
    # queue reaches it and never delays an exp. On transpose-heads (8, 10)
    # the slot must free before the mt=5 transpose: mm@1, sink@4.
    # qoco(p) overwrites qdup[:, p]: after head (p,0,1), before (p,1,0).
    pend_ps = {}

    def mm(key, fn, co):
        def run():
            pend_ps[key] = fn(co)
        return run

    def snk(key, fn, co):
        def run():
            fn(co, pend_ps.pop(key))
        return run

    sched = {
        # all 8 vmt tiles in the idx0 weave: their xv/wv DMA waits then
        # overlap the S/exp pipeline startup instead of stalling the
        # prologue (first AV reads vsb at idx1 step 0 — all emitted before)
        (0, 0): [lambda: vmt(0)], (0, 1): [lambda: vmt(4)],
        (0, 2): [lambda: vmt(1)], (0, 3): [lambda: vmt(5)],
        (0, 4): [lambda: vmt(2)], (0, 5): [lambda: vmt(6)],
        (0, 6): [lambda: vmt(3)], (0, 7): [lambda: vmt(7)],
        (1, 2): [mm("q1", qmm, 1)], (1, 7): [snk("q1", dup_sink, 1)],
        (2, 2): [mm("k1", kmm, 1)], (2, 7): [snk("k1", k_sink, 1)],
        (3, 2): [mm("o0", qomm, 0)], (3, 7): [snk("o0", dup_sink, 0)],
        (4, 2): [mm("q2", qmm, 2)], (4, 7): [snk("q2", dup_sink, 2)],
        (5, 2): [mm("o4", qomm, 4)], (5, 7): [snk("o4", dup_sink, 4)],
        (6, 2): [mm("k2", kmm, 2)], (6, 7): [snk("k2", k_sink, 2)],
        (7, 2): [mm("q3", qmm, 3)], (7, 7): [snk("q3", dup_sink, 3)],
        (8, 1): [mm("o1", qomm, 1)], (8, 4): [snk("o1", dup_sink, 1)],
        (9, 2): [mm("k3", kmm, 3)], (9, 7): [snk("k3", k_sink, 3)],
        (10, 1): [mm("q5", qmm, 5)], (10, 4): [snk("q5", dup_sink, 5)],
        (11, 2): [mm("k5", kmm, 5)], (11, 7): [snk("k5", k_sink, 5)],
        (13, 2): [mm("o2", qomm, 2)], (13, 7): [snk("o2", dup_sink, 2)],
        (17, 2): [mm("o3", qomm, 3)], (17, 7): [snk("o3", dup_sink, 3)],
        (20, 2): [mm("o5", qomm, 5)], (20, 7): [snk("o5", dup_sink, 5)],
    }

    pend = None
    for idx, (p, att, h) in enumerate(heads):
        a = emit_head(idx, p, att, h, pend, sched)
        pend = (p, att, h, a)
    o = psO.tile([P, NT, P], FP32, tag="o", name="o_av")
    pend = pend + (o,)
    for nt in range(NT):
        emit_av_slice(pend, nt)
    emit_norm(pend)
    emit_transpose(pend[0])

    # ---- phase 3: output projection (alternating psum slots) ---------------
    def proj_nt(nt, ps):
        for base, wd in ((0, 512), (512, 256)):
            for ct in range(CT):
                nc.tensor.matmul(
                    ps[:, base:base + wd],
                    onormT[:, ct, nt * P:(nt + 1) * P],
                    wp[:, ct, base:base + wd],
                    start=(ct == 0), stop=(ct == CT - 1))
        osb = opool.tile([P, C], BF16, tag="out", name="osb")
        nc.vector.tensor_copy(osb[:], ps[:, 0:C])
        nc.sync.dma_start(d_out[nt * P:(nt + 1) * P, :], osb[:])

    for nt in range(NT):
        if nt % 2 == 0:
            ps = psP.tile([P, N], FP32, tag="p", name="ps_proj")
        else:
            # borrow the (now idle) o-slot: same 4KB, reshaped flat
            ps = psO.tile([P, NT, P], FP32, tag="o",
                          name="ps_proj2").rearrange("p a b -> p (a b)")
        proj_nt(nt, ps)

    ctx.close()


_NC = None


def _get_nc():
    global _NC
    if _NC is None:
        _NC = build_kernel()
    return _NC


def prepare_in_maps(x, y, w_qkv, w_proj, b_proj):
    x = np.asarray(x, np.float32)
    y = np.asarray(y, np.float32)
    w_qkv = np.asarray(w_qkv, np.float32)
    w_proj = np.asarray(w_proj, np.float32)

    bf = ml_dtypes.bfloat16
    tb = lambda a: np.ascontiguousarray(a.T).astype(bf)
    wqo16 = tb(w_qkv[0:C])
    wq16 = tb(w_qkv[C:2 * C])
    wk16 = tb(w_qkv[2 * C:3 * C])
    wv16 = tb(w_qkv[3 * C:4 * C])
    wp16 = tb(w_proj)
    eye16 = np.eye(P, dtype=bf)

    in_maps = []
    for i in range(8):
        b = i % 4
        isx = i < 4
        t_qk = x[b] if isx else y[b]
        t_qo = y[b] if isx else x[b]
        in_maps.append({
            "qk16": tb(t_qk), "qo16": tb(t_qo), "v16": tb(x[b]),
            "wq16": wq16, "wk16": wk16, "wqo16": wqo16, "wv16": wv16,
            "wp16": wp16, "eye16": eye16,
        })
    return in_maps


def kernel(x, y, w_qkv, w_proj, b_proj):
    nc = _get_nc()
    in_maps = prepare_in_maps(x, y, w_qkv, w_proj, b_proj)
    res = run_bass_kernel_spmd(nc, in_maps, list(range(8)))
    bpf = np.asarray(b_proj, np.float32)
    out_x = np.stack([np.asarray(res.results[b]["out"], np.float32)
                      for b in range(4)]) + bpf
    out_y = np.stack([np.asarray(res.results[4 + b]["out"], np.float32)
                      for b in range(4)]) + bpf
    return out_x.astype(np.float32), out_y.astype(np.float32)


if __name__ == "__main__":
    rng = np.random.default_rng(0)
    ins = {
        "x": rng.standard_normal((B, N, C), dtype=np.float32),
        "y": rng.standard_normal((B, N, C), dtype=np.float32),
        "w_qkv": (rng.standard_normal((4 * C, C)) * 0.02).astype(np.float32),
        "w_proj": (rng.standard_normal((C, C)) * 0.02).astype(np.float32),
        "b_proj": (rng.standard_normal(C) * 0.02).astype(np.float32),
    }
    ox, oy = kernel(**ins)
    print(ox.shape, oy.shape, ox.dtype)


# revision 91
# speedup vs baseline: 1.0263x; 1.0263x over previous
"""Trainium2 Bass kernel for the dual-stream "DifAttention" block — v16.

Partitioning: 8 independent (batch, stream) units, one per core, SPMD, no
collectives:
    x-core b: t_qk=x[b], t_v=x[b], t_qo=y[b]
    y-core b: t_qk=y[b], t_v=x[b], t_qo=x[b]

Design (measured-model driven; ~277us median, rel err ~1.1e-2):

  projections      plain bf16, 12 matmuls per output col-tile (6144 cyc/co).
                   Measured: the v3 split-fp8 3-term DoubleRow form costs
                   9216 cyc/co — DR halves per-term cycles but the 3 terms
                   cost 1.5x bf16. 1-term fp8 DR is no faster either: the
                   512-col matmul pace (~250ns) is dispatch-bound, not ALU-
                   bound. bf16 is fastest AND most accurate here.
  S^T = K Q^T      bf16 blockdiag: stationary [128,128] = blockdiag(k[d,mA],
                   k[d,mB]), moving = q duplicated across partition halves.
                   512 cyc per [128m x 512n]; only bf16 cast error survives.
  exp split        ACT does 6 of 8 m-tiles per (head,att) via native EXP;
                   DVE does 2 of 8 via a Schraudolph bit-trick:
                   a = bitcast_bf16(int16(s*A + B)), A = +-0.125*log2e*128,
                   B = 128*(127 + c), c = -0.0564 (mean-zero centering: no
                   softmax-mass bias between DVE and ACT tiles). This keeps
                   the exp stream off the critical path (PE is the pacer).
  A V              o[n,d] form: stationary = A^T tile [128m x 128n] bf16,
                   moving = V[m, 64d + ones-col]; the softmax denominator
                   lands as a per-partition column. ~59ns/matmul issue rate
                   (dispatch floor — fp8 stationary does NOT help).
  out proj         bf16 from onorm^T (PE-transposed via identity matmuls).

Scheduling: per-c-tile input DMAs (contiguous source rows, ~4x faster than
a whole-tensor rearrange walk), explicit (head idx, weave mt) filler
schedule with deferred sinks (mm@2/sink@7) so projection casts are ready
when the in-order DVE queue reaches them; separate PSUM pools for s-tiles
(2 bufs), o (1), fillers/transposes/out-proj (1) = 8 banks exactly. The
prologue projections rotate through the (still idle) s/o psum slots so
they don't serialize behind each other's DVE casts; the output is stored
bf16 (tail is store-drain bound; b_proj added host-side in fp32).
"""

import numpy as np
import ml_dtypes

import concourse.bass as bass
import concourse.bacc as bacc
import concourse.tile as tile
from concourse import mybir
from concourse.bass_utils import run_bass_kernel_spmd

P = 128
B, N, C = 4, 1024, 768
H, HD = 12, 64
CT = C // P           # 6 column tiles (= head pairs)
NT = N // P           # 8 sequence tiles
EXPSC = 0.125         # 1/sqrt(hd)

LOG2E = 1.4426950408889634
C_CENTER = -0.056401  # mean-zero Schraudolph centering
EXP_A = EXPSC * LOG2E * 128.0
EXP_B = 128.0 * (127.0 + C_CENTER)
DVE_MTS = (2, 5)      # m-tiles per (head,att) handled by the DVE exp
# (1,4,6) with a third DVE tile measured time-equivalent within run
# variance but +0.7e-3 err — the depth-2 s-ring micro-stalls (~14us of
# sub-400ns gaps in the filler-light heads) are not drain-rate-bound.

FP32 = mybir.dt.float32
BF16 = mybir.dt.bfloat16
FP8 = mybir.dt.float8e4
I16 = mybir.dt.int16
EXP = mybir.ActivationFunctionType.Exp
MUL = mybir.AluOpType.mult
ADD = mybir.AluOpType.add


def build_kernel():
    nc = bacc.Bacc("TRN2", target_bir_lowering=False, debug=False,
                   num_devices=8)

    d_qk = nc.dram_tensor("qk16", [C, N], BF16, kind="ExternalInput")
    d_qo = nc.dram_tensor("qo16", [C, N], BF16, kind="ExternalInput")
    d_v = nc.dram_tensor("v16", [C, N], BF16, kind="ExternalInput")
    d_wq = nc.dram_tensor("wq16", [C, C], BF16, kind="ExternalInput")
    d_wk = nc.dram_tensor("wk16", [C, C], BF16, kind="ExternalInput")
    d_wqo = nc.dram_tensor("wqo16", [C, C], BF16, kind="ExternalInput")
    d_wv = nc.dram_tensor("wv16", [C, C], BF16, kind="ExternalInput")
    d_wp = nc.dram_tensor("wp16", [C, C], BF16, kind="ExternalInput")
    d_eye = nc.dram_tensor("eye16", [P, P], BF16, kind="ExternalInput")
    # bf16 output: halves the tail store traffic (the kernel end is
    # store-drain bound); b_proj is added host-side in fp32
    d_out = nc.dram_tensor("out", [N, C], BF16, kind="ExternalOutput")

    with tile.TileContext(nc) as tc:
        _body(tc, d_qk, d_qo, d_v, d_wq, d_wk, d_wqo, d_wv, d_wp, d_eye,
              d_out)
    nc.compile()
    return nc


def _body(tc, d_qk, d_qo, d_v, d_wq, d_wk, d_wqo, d_wv, d_wp, d_eye, d_out):
    nc = tc.nc
    _ap = lambda t: t if isinstance(t, bass.AP) else t.ap()
    d_qk, d_qo, d_v, d_wq, d_wk, d_wqo, d_wv, d_wp, d_eye, d_out = (
        _ap(t) for t in (d_qk, d_qo, d_v, d_wq, d_wk, d_wqo, d_wv, d_wp,
                         d_eye, d_out))
    from contextlib import ExitStack
    ctx = ExitStack()
    xpool = ctx.enter_context(tc.tile_pool(name="xpool", bufs=2))
    wpool = ctx.enter_context(tc.tile_pool(name="wpool", bufs=2))
    spool = ctx.enter_context(tc.tile_pool(name="spool", bufs=2))
    persist = ctx.enter_context(tc.tile_pool(name="persist", bufs=1))
    apool = ctx.enter_context(tc.tile_pool(name="apool", bufs=2))
    rpool = ctx.enter_context(tc.tile_pool(name="rpool", bufs=2))
    tpool = ctx.enter_context(tc.tile_pool(name="tpool", bufs=2))
    opool = ctx.enter_context(tc.tile_pool(name="opool", bufs=2))
    psS = ctx.enter_context(tc.tile_pool(name="psS", bufs=2, space="PSUM"))
    psO = ctx.enter_context(tc.tile_pool(name="psO", bufs=1, space="PSUM"))
    psP = ctx.enter_context(tc.tile_pool(name="psP", bufs=1, space="PSUM"))

    # ---- persistent tensors -------------------------------------------------
    # kblk[k, co, h, mt, M]: bf16 blockdiag stationary per (co,h,mt):
    #   rows 0-63 x cols 0-63 = k[d, mA], rows 64-127 x cols 64-127 = k[d, mB]
    # (fp8 kblk was time-neutral — S pacing is dispatch-bound — so bf16 buys
    # back ~3e-3 of error for free, funding the bf16 output store)
    kblk = persist.tile([P, CT, 2, NT, P], BF16, name="kblk")
    # q duplicated to both partition halves; per-co overwritten by qo after
    # the (co, att=0, *) heads have been emitted (emission order = dep order)
    qdup = persist.tile([P, CT, 2, N], BF16, name="qdup")
    # V[m, head, d] bf16 with a ones column at d=64 (softmax denominator)
    vsb = persist.tile([P, NT, H, HD + 1], BF16, name="vsb")
    onorm = persist.tile([P, NT, C], BF16, name="onorm")   # O[n, c]
    onormT = persist.tile([P, CT, N], BF16, name="onormT")  # O^T[c, n]
    eye = persist.tile([P, P], BF16, name="eye")

    # zero the off-diagonal blocks of kblk on DVE; one memset per
    # (co, partition-half) covers both h (contiguous nt-rows)
    def zmemset(co):
        for parts, coff in ((slice(0, 64), 64), (slice(64, P), 0)):
            base = kblk[parts, co, 0, 0, coff:coff + 64]
            dst = bass.AP(tensor=base.tensor, offset=base.offset,
                          ap=[list(base.ap[0]), [P, 2 * NT], [1, 64]])
            nc.vector.memset(dst, 0.0)

    # ---- load inputs --------------------------------------------------------
    # Per-c-tile DMAs: each reads 128 CONTIGUOUS source rows (the whole-tensor
    # rearrange walks the source p-major = 768 scattered row reads ~79GB/s;
    # split loads measured ~4x faster)
    def load_split(dst, d_src, eng):
        for t in range(CT):
            eng.dma_start(dst[:, t, :], d_src[t * P:(t + 1) * P, :])

    xqk = xpool.tile([P, CT, N], BF16, tag="x16", name="xqk")
    xqo = xpool.tile([P, CT, N], BF16, tag="x16", name="xqo")
    xv = xpool.tile([P, CT, N], BF16, tag="x16b", name="xv", bufs=1)
    load_split(xqk, d_qk, nc.sync)
    nc.sync.dma_start(eye[:], d_eye)
    # xv split across two rings so it lands ~4us earlier for the vmt fillers
    for t in range(3):
        nc.sync.dma_start(xv[:, t, :], d_v[t * P:(t + 1) * P, :])

    # ---- phase 1: projections (bf16), co-granular. The matmul part and the
    # sink (cast + scatter) are scheduled separately: the sink's DVE cast
    # sits in the same in-order queue as the exp stream, so it must only be
    # reached once its psum is long finished ---------------------------------
    def make_proj(d_w, srcx, name, eng=None):
        wsb = wpool.tile([P, CT, C], BF16, tag="w16", name=name, bufs=3)
        load_split(wsb, d_w, eng or nc.gpsimd)

        def mm_fn(co, ps=None):
            if ps is None:
                ps = psP.tile([P, N], FP32, tag="p", name="ps_qkv")
            cosl = slice(co * P, (co + 1) * P)
            for ch in range(2):
                nsl = slice(ch * 512, (ch + 1) * 512)
                for j in range(CT):
                    nc.tensor.matmul(
                        ps[:, nsl], wsb[:, j, cosl], srcx[:, j, nsl],
                        start=(j == 0), stop=(j == CT - 1))
            return ps
        return mm_fn

    def dup_sink(co, ps):
        stg = spool.tile([P, N], BF16, tag="stg", name="qstg")
        nc.vector.tensor_copy(stg[:], ps[:])
        nc.gpsimd.dma_start(qdup[0:64, co, 0, :], stg[0:64, :])
        nc.gpsimd.dma_start(qdup[64:P, co, 0, :], stg[0:64, :])
        nc.gpsimd.dma_start(qdup[0:64, co, 1, :], stg[64:P, :])
        nc.gpsimd.dma_start(qdup[64:P, co, 1, :], stg[64:P, :])

    # ramp variant: cast + scatter per 512-col half, so the first S matmuls
    # can start as soon as the first half of the co=0 projection lands
    def dup_sink_split(co, ps):
        stg = spool.tile([P, N], BF16, tag="stg", name="qstg")
        for hf in range(2):
            nsl = slice(hf * 512, (hf + 1) * 512)
            nc.vector.tensor_copy(stg[:, nsl], ps[:, nsl])
            nc.gpsimd.dma_start(qdup[0:64, co, 0, nsl], stg[0:64, nsl])
            nc.gpsimd.dma_start(qdup[64:P, co, 0, nsl], stg[0:64, nsl])
            nc.gpsimd.dma_start(qdup[0:64, co, 1, nsl], stg[64:P, nsl])
            nc.gpsimd.dma_start(qdup[64:P, co, 1, nsl], stg[64:P, nsl])

    def _k_scatter(co, stg, mts):
        def stg_ap(prt, half):
            s = stg[prt, mts.start * P + half * 64:
                    mts.start * P + half * 64 + 64]
            return bass.AP(tensor=s.tensor, offset=s.offset,
                           ap=[list(s.ap[0]), [P, mts.stop - mts.start],
                               [1, 64]])
        nc.gpsimd.dma_start(kblk[0:64, co, 0, mts, 0:64],
                            stg_ap(slice(0, 64), 0))
        nc.gpsimd.dma_start(kblk[64:P, co, 0, mts, 64:P],
                            stg_ap(slice(0, 64), 1))
        nc.gpsimd.dma_start(kblk[0:64, co, 1, mts, 0:64],
                            stg_ap(slice(64, P), 0))
        nc.gpsimd.dma_start(kblk[64:P, co, 1, mts, 64:P],
                            stg_ap(slice(64, P), 1))

    # k stages through fp8: halves the scatter bytes on the gpsimd ring
    # (which also carries the q dups that gate the S matmuls); the k noise
    # (~1.8% rms) costs ~3e-3 output err — measured worth the ~3us
    def k_sink(co, ps):
        stg = spool.tile([P, N], FP8, tag="stg8", name="kstg")
        nc.vector.tensor_copy(stg[:], ps[:])
        _k_scatter(co, stg, slice(0, NT))

    def k_sink_split(co, ps):
        stg = spool.tile([P, N], FP8, tag="stg8", name="kstg")
        for hf in range(2):
            nsl = slice(hf * 512, (hf + 1) * 512)
            nc.vector.tensor_copy(stg[:, nsl], ps[:, nsl])
            _k_scatter(co, stg, slice(hf * 4, hf * 4 + 4))

    def make_vproj():
        wsb = wpool.tile([P, CT, C], BF16, tag="w16", name="wv16", bufs=3)
        load_split(wsb, d_wv, nc.gpsimd)

        def mt_fn(mt, ps=None):
            if ps is None:
                ps = psP.tile([P, N], FP32, tag="p", name="ps_v")
            msl = slice(mt * P, (mt + 1) * P)
            for base, wd in ((0, 512), (512, 256)):
                for j in range(CT):
                    nc.tensor.matmul(
                        ps[:, base:base + wd],
                        xv[:, j, msl], wsb[:, j, base:base + wd],
                        start=(j == 0), stop=(j == CT - 1))
            nc.vector.tensor_copy(
                vsb[:, mt, :, 0:HD],
                ps[:, 0:C].rearrange("p (h d) -> p h d", h=H))
        return mt_fn

    # ---- phase 2: attention, pipelined at (p, att, head) granularity --------
    # AV slices of the previous head are woven uniformly (1 per weave step,
    # 2 at step 6) so each step carries ~1us of PE work to match the exp
    # drain cadence; norm at step 6 still frees o a step before the next
    # head's o allocation needs it (psO has 1 buf).
    AV_SCHED = {0: (0,), 1: (1,), 2: (2,), 3: (3,), 4: (4,), 5: (5,),
                6: (6, 7)}

    def emit_av_slice(pend, nt):
        pp, patt, ph, a, o = pend
        hh = 2 * pp + ph
        ntsl = slice(nt * P, (nt + 1) * P)
        for mt in range(NT):
            nc.tensor.matmul(
                o[:, nt, 0:HD + 1], a[:, mt, ntsl],
                vsb[:, mt, hh, :],
                start=(mt == 0), stop=(mt == NT - 1),
                skip_group_check=True)

    def emit_norm(pend):
        pp, patt, ph, a, o = pend
        hh = 2 * pp + ph
        r = rpool.tile([P, NT, 1], FP32, tag="r", name="r_den")
        nc.vector.reciprocal(r[:], o[:, :, HD:HD + 1])
        rb = bass.AP(tensor=r.tensor, offset=r[:].offset,
                     ap=[list(r[:].ap[0]), [1, NT], [0, HD]])
        dst = onorm[:, :, hh * HD:(hh + 1) * HD]
        if patt == 0:
            nc.vector.tensor_mul(dst, o[:, :, 0:HD], rb)
        else:
            t = tpool.tile([P, NT, HD], BF16, tag="t", name="t_norm")
            nc.vector.tensor_mul(t[:], o[:, :, 0:HD], rb)
            nc.vector.tensor_add(dst, dst, t[:])

    def emit_transpose(p):
        trp = psP.tile([P, N], BF16, tag="p", name="tr")
        for nt in range(NT):
            nc.tensor.transpose(trp[:, nt * P:(nt + 1) * P],
                                onorm[:, nt, p * P:(p + 1) * P], eye[:])
        nc.vector.tensor_copy(onormT[:, p, :], trp[:])

    def emit_head(idx, p, att, h, pend, sched):
        sgn = 1.0 if att == 0 else -1.0
        a = apool.tile([P, NT, N], BF16, tag="a", name="a_att")
        if pend is not None:
            o = psO.tile([P, NT, P], FP32, tag="o", name="o_av")
            pend = pend + (o,)
        for mt in range(NT):
            # in the filler-light back-half heads the psP slot is idle;
            # borrowing it for one s-tile per head gives the depth-2 s-ring
            # a mid-unit relief point (PE can run one step further ahead of
            # the exp drain, cutting the per-step lockstep stalls)
            if (mt == 4 and idx in (12, 14, 15, 16, 18, 19, 21, 22, 23)) \
                    or (mt == 6 and idx in (12, 15, 16, 19, 21, 23)):
                s = psP.tile([P, N], FP32, tag="p", name="s_extra")
            else:
                s = psS.tile([P, N], FP32, tag="s", name="s_att")
            for ch in range(2):
                nsl = slice(ch * 512, (ch + 1) * 512)
                nc.tensor.matmul(
                    s[:, nsl], kblk[:, p, h, mt, :], qdup[:, p, h, nsl],
                    start=True, stop=True)
            if mt in DVE_MTS:
                nc.vector.tensor_scalar(
                    a[:, mt, :].bitcast(I16), s[:],
                    sgn * EXP_A, EXP_B, MUL, ADD)
            else:
                nc.scalar.activation(a[:, mt, :], s[:], EXP,
                                     scale=sgn * EXPSC)
            if pend is not None:
                for nt in AV_SCHED.get(mt, ()):
                    emit_av_slice(pend, nt)
                if mt == 6:
                    emit_norm(pend)
                    if pend[1] == 1 and pend[2] == 1:
                        emit_transpose(pend[0])
            for fn in sched.get((idx, mt), ()):
                fn()
        return a

    # weight loads: wq then xqo on the scalar queue, wqo on sync (its
    # w16-ring slot frees only after the vmt fillers, so its DMA must not
    # block the gpsimd scatters or the first exps), the rest on gpsimd
    kmm = make_proj(d_wk, xqk, "wk16")
    qmm = make_proj(d_wq, xqk, "wq16", eng=nc.scalar)
    for t in range(3, CT):
        nc.scalar.dma_start(xv[:, t, :], d_v[t * P:(t + 1) * P, :])
    load_split(xqo, d_qo, nc.scalar)
    vmt = make_vproj()
    qomm = make_proj(d_wqo, xqo, "wqo16", eng=nc.sync)

    # prologue: zeros + co 0 and 4 of Q/K inline (the PE would otherwise
    # stall on input DMA here anyway), vsb tiles 0-3 before the first AV.
    # The attention-phase psum slots (psS "s", psO "o") are still free here,
    # so rotate the prologue projections through them — a single psP slot
    # would serialize each co_fn behind the previous one's DVE cast.
    def pro_ps():
        i = 0
        while True:
            yield psP.tile([P, N], FP32, tag="p", name="ps_pro")
            yield psS.tile([P, N], FP32, tag="s", name="ps_pro2")
            yield psO.tile([P, NT, P], FP32, tag="o",
                           name="ps_pro3").rearrange("p a b -> p (a b)")
            i += 1
    pro = pro_ps()
    for co in range(CT):
        zmemset(co)
    # q first (wq's ring is shorter, so it lands before wk finishes), and
    # half-granular sinks so the first S matmuls start ~3us earlier
    dup_sink_split(0, qmm(0, next(pro)))
    k_sink_split(0, kmm(0, next(pro)))
    nc.vector.memset(vsb[:, :, :, HD:HD + 1], 1.0)
    # co=4 before the vmt tiles: the vmt matmuls wait on the xv DMA anyway,
    # and this keeps the PE busy through that window
    k_sink(4, kmm(4, next(pro)))
    dup_sink(4, qmm(4, next(pro)))
    for mt in range(4):
        vmt(mt, next(pro))
    wp = wpool.tile([P, CT, C], BF16, tag="wf", name="wp", bufs=1)
    load_split(wp, d_wp, nc.gpsimd)

    heads = [(0, 0, 0), (0, 0, 1), (4, 0, 0), (4, 0, 1), (1, 0, 0),
             (1, 0, 1), (0, 1, 0), (0, 1, 1), (4, 1, 0), (4, 1, 1),
             (2, 0, 0), (2, 0, 1), (1, 1, 0), (1, 1, 1), (3, 0, 0),
             (3, 0, 1), (2, 1, 0), (2, 1, 1), (5, 0, 0), (5, 0, 1),
             (3, 1, 0), (3, 1, 1), (5, 1, 0), (5, 1, 1)]
    # Explicit filler schedule, (head idx, weave mt) -> work. Each projection
    # emits its matmuls early in a head (mt=2) and its sink (DVE cast +
    # gpsimd scatter) late (mt=7), so the cast is ready by the time the DVE
    # queue reaches it and never delays an exp. On transpose-heads (8, 10)
    # the slot must free before the mt=5 transpose: mm@1, sink@4.
    # qoco(p) overwrites qdup[:, p]: after head (p,0,1), before (p,1,0).
    pend_ps = {}

    def mm(key, fn, co):
        def run():
            pend_ps[key] = fn(co)
        return run

    def snk(key, fn, co):
        def run():
            fn(co, pend_ps.pop(key))
        return run

    sched = {
        (0, 1): [lambda: vmt(4)], (0, 3): [lambda: vmt(5)],
        (0, 5): [lambda: vmt(6)], (0, 7): [lambda: vmt(7)],
        (1, 2): [mm("q1", qmm, 1)], (1, 7): [snk("q1", dup_sink, 1)],
        (2, 2): [mm("k1", kmm, 1)], (2, 7): [snk("k1", k_sink, 1)],
        (3, 2): [mm("o0", qomm, 0)], (3, 7): [snk("o0", dup_sink, 0)],
        (4, 2): [mm("q2", qmm, 2)], (4, 7): [snk("q2", dup_sink, 2)],
        (5, 2): [mm("o4", qomm, 4)], (5, 7): [snk("o4", dup_sink, 4)],
        (6, 2): [mm("k2", kmm, 2)], (6, 7): [snk("k2", k_sink, 2)],
        (7, 2): [mm("q3", qmm, 3)], (7, 7): [snk("q3", dup_sink, 3)],
        (8, 1): [mm("o1", qomm, 1)], (8, 4): [snk("o1", dup_sink, 1)],
        (9, 2): [mm("k3", kmm, 3)], (9, 7): [snk("k3", k_sink, 3)],
        (10, 1): [mm("q5", qmm, 5)], (10, 4): [snk("q5", dup_sink, 5)],
        (11, 2): [mm("k5", kmm, 5)], (11, 7): [snk("k5", k_sink, 5)],
        (13, 2): [mm("o2", qomm, 2)], (13, 7): [snk("o2", dup_sink, 2)],
        (17, 2): [mm("o3", qomm, 3)], (17, 7): [snk("o3", dup_sink, 3)],
        (20, 2): [mm("o5", qomm, 5)], (20, 7): [snk("o5", dup_sink, 5)],
    }

    pend = None
    for idx, (p, att, h) in enumerate(heads):
        a = emit_head(idx, p, att, h, pend, sched)
        pend = (p, att, h, a)
    o = psO.tile([P, NT, P], FP32, tag="o", name="o_av")
    pend = pend + (o,)
    for nt in range(NT):
        emit_av_slice(pend, nt)
    emit_norm(pend)
    emit_transpose(pend[0])

    # ---- phase 3: output projection (alternating psum slots) ---------------
    def proj_nt(nt, ps):
        for base, wd in ((0, 512), (512, 256)):
            for ct in range(CT):
                nc.tensor.matmul(
                    ps[:, base:base + wd],
                    onormT[:, ct, nt * P:(nt + 1) * P],
                    wp[:, ct, base:base + wd],
                    start=(ct == 0), stop=(ct == CT - 1))
        osb = opool.tile([P, C], BF16, tag="out", name="osb")
        nc.vector.tensor_copy(osb[:], ps[:, 0:C])
        nc.sync.dma_start(d_out[nt * P:(nt + 1) * P, :], osb[:])

    for nt in range(NT):
        if nt % 2 == 0:
            ps = psP.tile([P, N], FP32, tag="p", name="ps_proj")
        else:
            # borrow the (now idle) o-slot: same 4KB, reshaped flat
            ps = psO.tile([P, NT, P], FP32, tag="o",
                          name="ps_proj2").rearrange("p a b -> p (a b)")
        proj_nt(nt, ps)

    ctx.close()


_NC = None


def _get_nc():
    global _NC
    if _NC is None:
        _NC = build_kernel()
    return _NC


def prepare_in_maps(x, y, w_qkv, w_proj, b_proj):
    x = np.asarray(x, np.float32)
    y = np.asarray(y, np.float32)
    w_qkv = np.asarray(w_qkv, np.float32)
    w_proj = np.asarray(w_proj, np.float32)

    bf = ml_dtypes.bfloat16
    tb = lambda a: np.ascontiguousarray(a.T).astype(bf)
    wqo16 = tb(w_qkv[0:C])
    wq16 = tb(w_qkv[C:2 * C])
    wk16 = tb(w_qkv[2 * C:3 * C])
    wv16 = tb(w_qkv[3 * C:4 * C])
    wp16 = tb(w_proj)
    eye16 = np.eye(P, dtype=bf)

    in_maps = []
    for i in range(8):
        b = i % 4
        isx = i < 4
        t_qk = x[b] if isx else y[b]
        t_qo = y[b] if isx else x[b]
        in_maps.append({
            "qk16": tb(t_qk), "qo16": tb(t_qo), "v16": tb(x[b]),
            "wq16": wq16, "wk16": wk16, "wqo16": wqo16, "wv16": wv16,
            "wp16": wp16, "eye16": eye16,
        })
    return in_maps


def kernel(x, y, w_qkv, w_proj, b_proj):
    nc = _get_nc()
    in_maps = prepare_in_maps(x, y, w_qkv, w_proj, b_proj)
    res = run_bass_kernel_spmd(nc, in_maps, list(range(8)))
    bpf = np.asarray(b_proj, np.float32)
    out_x = np.stack([np.asarray(res.results[b]["out"], np.float32)
                      for b in range(4)]) + bpf
    out_y = np.stack([np.asarray(res.results[4 + b]["out"], np.float32)
                      for b in range(4)]) + bpf
    return out_x.astype(np.float32), out_y.astype(np.float32)


if __name__ == "__main__":
    rng = np.random.default_rng(0)
    ins = {
        "x": rng.standard_normal((B, N, C), dtype=np.float32),
        "y": rng.standard_normal((B, N, C), dtype=np.float32),
        "w_qkv": (rng.standard_normal((4 * C, C)) * 0.02).astype(np.float32),
        "w_proj": (rng.standard_normal((C, C)) * 0.02).astype(np.float32),
        "b_proj": (rng.standard_normal(C) * 0.02).astype(np.float32),
    }
    ox, oy = kernel(**ins)
    print(ox.shape, oy.shape, ox.dtype)


# revision 93
# speedup vs baseline: 1.0351x; 1.0086x over previous
"""Trainium2 Bass kernel for the dual-stream "DifAttention" block — v16.

Partitioning: 8 independent (batch, stream) units, one per core, SPMD, no
collectives:
    x-core b: t_qk=x[b], t_v=x[b], t_qo=y[b]
    y-core b: t_qk=y[b], t_v=x[b], t_qo=x[b]

Design (measured-model driven; ~277us median, rel err ~1.1e-2):

  projections      plain bf16, 12 matmuls per output col-tile (6144 cyc/co).
                   Measured: the v3 split-fp8 3-term DoubleRow form costs
                   9216 cyc/co — DR halves per-term cycles but the 3 terms
                   cost 1.5x bf16. 1-term fp8 DR is no faster either: the
                   512-col matmul pace (~250ns) is dispatch-bound, not ALU-
                   bound. bf16 is fastest AND most accurate here.
  S^T = K Q^T      bf16 blockdiag: stationary [128,128] = blockdiag(k[d,mA],
                   k[d,mB]), moving = q duplicated across partition halves.
                   512 cyc per [128m x 512n]; only bf16 cast error survives.
  exp split        ACT does 6 of 8 m-tiles per (head,att) via native EXP;
                   DVE does 2 of 8 via a Schraudolph bit-trick:
                   a = bitcast_bf16(int16(s*A + B)), A = +-0.125*log2e*128,
                   B = 128*(127 + c), c = -0.0564 (mean-zero centering: no
                   softmax-mass bias between DVE and ACT tiles). This keeps
                   the exp stream off the critical path (PE is the pacer).
  A V              o[n,d] form: stationary = A^T tile [128m x 128n] bf16,
                   moving = V[m, 64d + ones-col]; the softmax denominator
                   lands as a per-partition column. ~59ns/matmul issue rate
                   (dispatch floor — fp8 stationary does NOT help).
  out proj         bf16 from onorm^T (PE-transposed via identity matmuls).

Scheduling: per-c-tile input DMAs (contiguous source rows, ~4x faster than
a whole-tensor rearrange walk), explicit (head idx, weave mt) filler
schedule with deferred sinks (mm@2/sink@7) so projection casts are ready
when the in-order DVE queue reaches them; separate PSUM pools for s-tiles
(2 bufs), o (1), fillers/transposes/out-proj (1) = 8 banks exactly. The
prologue projections rotate through the (still idle) s/o psum slots so
they don't serialize behind each other's DVE casts; the output is stored
bf16 (tail is store-drain bound; b_proj added host-side in fp32).
"""

import numpy as np
import ml_dtypes

import concourse.bass as bass
import concourse.bacc as bacc
import concourse.tile as tile
from concourse import mybir
from concourse.bass_utils import run_bass_kernel_spmd

P = 128
B, N, C = 4, 1024, 768
H, HD = 12, 64
CT = C // P           # 6 column tiles (= head pairs)
NT = N // P           # 8 sequence tiles
EXPSC = 0.125         # 1/sqrt(hd)

LOG2E = 1.4426950408889634
C_CENTER = -0.056401  # mean-zero Schraudolph centering
EXP_A = EXPSC * LOG2E * 128.0
EXP_B = 128.0 * (127.0 + C_CENTER)
DVE_MTS = (2, 5)      # m-tiles per (head,att) handled by the DVE exp
# (1,4,6) with a third DVE tile measured time-equivalent within run
# variance but +0.7e-3 err — the depth-2 s-ring micro-stalls (~14us of
# sub-400ns gaps in the filler-light heads) are not drain-rate-bound.

FP32 = mybir.dt.float32
BF16 = mybir.dt.bfloat16
FP8 = mybir.dt.float8e4
I16 = mybir.dt.int16
EXP = mybir.ActivationFunctionType.Exp
MUL = mybir.AluOpType.mult
ADD = mybir.AluOpType.add


def build_kernel():
    nc = bacc.Bacc("TRN2", target_bir_lowering=False, debug=False,
                   num_devices=8)

    d_qk = nc.dram_tensor("qk16", [C, N], BF16, kind="ExternalInput")
    d_qo = nc.dram_tensor("qo16", [C, N], BF16, kind="ExternalInput")
    d_v = nc.dram_tensor("v16", [C, N], BF16, kind="ExternalInput")
    d_wq = nc.dram_tensor("wq16", [C, C], BF16, kind="ExternalInput")
    d_wk = nc.dram_tensor("wk16", [C, C], BF16, kind="ExternalInput")
    d_wqo = nc.dram_tensor("wqo16", [C, C], BF16, kind="ExternalInput")
    d_wv = nc.dram_tensor("wv16", [C, C], BF16, kind="ExternalInput")
    d_wp = nc.dram_tensor("wp16", [C, C], BF16, kind="ExternalInput")
    d_eye = nc.dram_tensor("eye16", [P, P], BF16, kind="ExternalInput")
    # bf16 output: halves the tail store traffic (the kernel end is
    # store-drain bound); b_proj is added host-side in fp32
    d_out = nc.dram_tensor("out", [N, C], BF16, kind="ExternalOutput")

    with tile.TileContext(nc) as tc:
        _body(tc, d_qk, d_qo, d_v, d_wq, d_wk, d_wqo, d_wv, d_wp, d_eye,
              d_out)
    nc.compile()
    return nc


def _body(tc, d_qk, d_qo, d_v, d_wq, d_wk, d_wqo, d_wv, d_wp, d_eye, d_out):
    nc = tc.nc
    _ap = lambda t: t if isinstance(t, bass.AP) else t.ap()
    d_qk, d_qo, d_v, d_wq, d_wk, d_wqo, d_wv, d_wp, d_eye, d_out = (
        _ap(t) for t in (d_qk, d_qo, d_v, d_wq, d_wk, d_wqo, d_wv, d_wp,
                         d_eye, d_out))
    from contextlib import ExitStack
    ctx = ExitStack()
    xpool = ctx.enter_context(tc.tile_pool(name="xpool", bufs=2))
    wpool = ctx.enter_context(tc.tile_pool(name="wpool", bufs=2))
    spool = ctx.enter_context(tc.tile_pool(name="spool", bufs=2))
    persist = ctx.enter_context(tc.tile_pool(name="persist", bufs=1))
    apool = ctx.enter_context(tc.tile_pool(name="apool", bufs=2))
    rpool = ctx.enter_context(tc.tile_pool(name="rpool", bufs=2))
    tpool = ctx.enter_context(tc.tile_pool(name="tpool", bufs=2))
    opool = ctx.enter_context(tc.tile_pool(name="opool", bufs=2))
    psS = ctx.enter_context(tc.tile_pool(name="psS", bufs=2, space="PSUM"))
    psO = ctx.enter_context(tc.tile_pool(name="psO", bufs=1, space="PSUM"))
    psP = ctx.enter_context(tc.tile_pool(name="psP", bufs=1, space="PSUM"))

    # ---- persistent tensors -------------------------------------------------
    # kblk[k, co, h, mt, M]: bf16 blockdiag stationary per (co,h,mt):
    #   rows 0-63 x cols 0-63 = k[d, mA], rows 64-127 x cols 64-127 = k[d, mB]
    # (fp8 kblk was time-neutral — S pacing is dispatch-bound — so bf16 buys
    # back ~3e-3 of error for free, funding the bf16 output store)
    kblk = persist.tile([P, CT, 2, NT, P], BF16, name="kblk")
    # q duplicated to both partition halves; per-co overwritten by qo after
    # the (co, att=0, *) heads have been emitted (emission order = dep order)
    qdup = persist.tile([P, CT, 2, N], BF16, name="qdup")
    # V[m, head, d] bf16 with a ones column at d=64 (softmax denominator)
    vsb = persist.tile([P, NT, H, HD + 1], BF16, name="vsb")
    onorm = persist.tile([P, NT, C], BF16, name="onorm")   # O[n, c]
    onormT = persist.tile([P, CT, N], BF16, name="onormT")  # O^T[c, n]
    eye = persist.tile([P, P], BF16, name="eye")

    # zero the off-diagonal blocks of kblk on DVE; one memset per
    # (co, partition-half) covers both h (contiguous nt-rows)
    def zmemset(co):
        for parts, coff in ((slice(0, 64), 64), (slice(64, P), 0)):
            base = kblk[parts, co, 0, 0, coff:coff + 64]
            dst = bass.AP(tensor=base.tensor, offset=base.offset,
                          ap=[list(base.ap[0]), [P, 2 * NT], [1, 64]])
            nc.vector.memset(dst, 0.0)

    # ---- load inputs --------------------------------------------------------
    # Per-c-tile DMAs: each reads 128 CONTIGUOUS source rows (the whole-tensor
    # rearrange walks the source p-major = 768 scattered row reads ~79GB/s;
    # split loads measured ~4x faster)
    def load_split(dst, d_src, eng):
        for t in range(CT):
            eng.dma_start(dst[:, t, :], d_src[t * P:(t + 1) * P, :])

    xqk = xpool.tile([P, CT, N], BF16, tag="x16", name="xqk")
    xqo = xpool.tile([P, CT, N], BF16, tag="x16", name="xqo")
    xv = xpool.tile([P, CT, N], BF16, tag="x16b", name="xv", bufs=1)
    load_split(xqk, d_qk, nc.sync)
    nc.sync.dma_start(eye[:], d_eye)
    # xv split across two rings so it lands ~4us earlier for the vmt fillers
    for t in range(3):
        nc.sync.dma_start(xv[:, t, :], d_v[t * P:(t + 1) * P, :])

    # ---- phase 1: projections (bf16), co-granular. The matmul part and the
    # sink (cast + scatter) are scheduled separately: the sink's DVE cast
    # sits in the same in-order queue as the exp stream, so it must only be
    # reached once its psum is long finished ---------------------------------
    def make_proj(d_w, srcx, name, eng=None):
        wsb = wpool.tile([P, CT, C], BF16, tag="w16", name=name, bufs=3)
        load_split(wsb, d_w, eng or nc.gpsimd)

        def mm_fn(co, ps=None):
            if ps is None:
                ps = psP.tile([P, N], FP32, tag="p", name="ps_qkv")
            cosl = slice(co * P, (co + 1) * P)
            for ch in range(2):
                nsl = slice(ch * 512, (ch + 1) * 512)
                for j in range(CT):
                    nc.tensor.matmul(
                        ps[:, nsl], wsb[:, j, cosl], srcx[:, j, nsl],
                        start=(j == 0), stop=(j == CT - 1))
            return ps
        return mm_fn

    def dup_sink(co, ps):
        stg = spool.tile([P, N], BF16, tag="stg", name="qstg")
        nc.vector.tensor_copy(stg[:], ps[:])
        nc.gpsimd.dma_start(qdup[0:64, co, 0, :], stg[0:64, :])
        nc.gpsimd.dma_start(qdup[64:P, co, 0, :], stg[0:64, :])
        nc.gpsimd.dma_start(qdup[0:64, co, 1, :], stg[64:P, :])
        nc.gpsimd.dma_start(qdup[64:P, co, 1, :], stg[64:P, :])

    # ramp variant: cast + scatter per 512-col half, so the first S matmuls
    # can start as soon as the first half of the co=0 projection lands
    def dup_sink_split(co, ps):
        stg = spool.tile([P, N], BF16, tag="stg", name="qstg")
        for hf in range(2):
            nsl = slice(hf * 512, (hf + 1) * 512)
            nc.vector.tensor_copy(stg[:, nsl], ps[:, nsl])
            nc.gpsimd.dma_start(qdup[0:64, co, 0, nsl], stg[0:64, nsl])
            nc.gpsimd.dma_start(qdup[64:P, co, 0, nsl], stg[0:64, nsl])
            nc.gpsimd.dma_start(qdup[0:64, co, 1, nsl], stg[64:P, nsl])
            nc.gpsimd.dma_start(qdup[64:P, co, 1, nsl], stg[64:P, nsl])

    def _k_scatter(co, stg, mts):
        def stg_ap(prt, half):
            s = stg[prt, mts.start * P + half * 64:
                    mts.start * P + half * 64 + 64]
            return bass.AP(tensor=s.tensor, offset=s.offset,
                           ap=[list(s.ap[0]), [P, mts.stop - mts.start],
                               [1, 64]])
        nc.gpsimd.dma_start(kblk[0:64, co, 0, mts, 0:64],
                            stg_ap(slice(0, 64), 0))
        nc.gpsimd.dma_start(kblk[64:P, co, 0, mts, 64:P],
                            stg_ap(slice(0, 64), 1))
        nc.gpsimd.dma_start(kblk[0:64, co, 1, mts, 0:64],
                            stg_ap(slice(64, P), 0))
        nc.gpsimd.dma_start(kblk[64:P, co, 1, mts, 64:P],
                            stg_ap(slice(64, P), 1))

    # k stages through fp8: halves the scatter bytes on the gpsimd ring
    # (which also carries the q dups that gate the S matmuls); the k noise
    # (~1.8% rms) costs ~3e-3 output err — measured worth the ~3us
    def k_sink(co, ps):
        stg = spool.tile([P, N], FP8, tag="stg8", name="kstg")
        nc.vector.tensor_copy(stg[:], ps[:])
        _k_scatter(co, stg, slice(0, NT))

    def k_sink_split(co, ps):
        stg = spool.tile([P, N], FP8, tag="stg8", name="kstg")
        for hf in range(2):
            nsl = slice(hf * 512, (hf + 1) * 512)
            nc.vector.tensor_copy(stg[:, nsl], ps[:, nsl])
            _k_scatter(co, stg, slice(hf * 4, hf * 4 + 4))

    def make_vproj():
        wsb = wpool.tile([P, CT, C], BF16, tag="w16", name="wv16", bufs=3)
        load_split(wsb, d_wv, nc.gpsimd)

        def mt_fn(mt, ps=None):
            if ps is None:
                ps = psP.tile([P, N], FP32, tag="p", name="ps_v")
            msl = slice(mt * P, (mt + 1) * P)
            for base, wd in ((0, 512), (512, 256)):
                for j in range(CT):
                    nc.tensor.matmul(
                        ps[:, base:base + wd],
                        xv[:, j, msl], wsb[:, j, base:base + wd],
                        start=(j == 0), stop=(j == CT - 1))
            nc.vector.tensor_copy(
                vsb[:, mt, :, 0:HD],
                ps[:, 0:C].rearrange("p (h d) -> p h d", h=H))
        return mt_fn

    # ---- phase 2: attention, pipelined at (p, att, head) granularity --------
    # AV slices of the previous head are woven uniformly (1 per weave step,
    # 2 at step 6) so each step carries ~1us of PE work to match the exp
    # drain cadence; norm at step 6 still frees o a step before the next
    # head's o allocation needs it (psO has 1 buf).
    AV_SCHED = {0: (0,), 1: (1,), 2: (2,), 3: (3,), 4: (4,), 5: (5,),
                6: (6, 7)}

    def emit_av_slice(pend, nt):
        pp, patt, ph, a, o = pend
        hh = 2 * pp + ph
        ntsl = slice(nt * P, (nt + 1) * P)
        for mt in range(NT):
            nc.tensor.matmul(
                o[:, nt, 0:HD + 1], a[:, mt, ntsl],
                vsb[:, mt, hh, :],
                start=(mt == 0), stop=(mt == NT - 1),
                skip_group_check=True)

    def emit_norm(pend):
        pp, patt, ph, a, o = pend
        hh = 2 * pp + ph
        r = rpool.tile([P, NT, 1], FP32, tag="r", name="r_den")
        nc.vector.reciprocal(r[:], o[:, :, HD:HD + 1])
        rb = bass.AP(tensor=r.tensor, offset=r[:].offset,
                     ap=[list(r[:].ap[0]), [1, NT], [0, HD]])
        dst = onorm[:, :, hh * HD:(hh + 1) * HD]
        if patt == 0:
            nc.vector.tensor_mul(dst, o[:, :, 0:HD], rb)
        else:
            t = tpool.tile([P, NT, HD], BF16, tag="t", name="t_norm")
            nc.vector.tensor_mul(t[:], o[:, :, 0:HD], rb)
            nc.vector.tensor_add(dst, dst, t[:])

    def emit_transpose(p):
        trp = psP.tile([P, N], BF16, tag="p", name="tr")
        for nt in range(NT):
            nc.tensor.transpose(trp[:, nt * P:(nt + 1) * P],
                                onorm[:, nt, p * P:(p + 1) * P], eye[:])
        nc.vector.tensor_copy(onormT[:, p, :], trp[:])

    def emit_head(idx, p, att, h, pend, sched):
        sgn = 1.0 if att == 0 else -1.0
        a = apool.tile([P, NT, N], BF16, tag="a", name="a_att")
        if pend is not None:
            o = psO.tile([P, NT, P], FP32, tag="o", name="o_av")
            pend = pend + (o,)
        for mt in range(NT):
            # in the filler-light back-half heads the psP slot is idle;
            # borrowing it for one s-tile per head gives the depth-2 s-ring
            # a mid-unit relief point (PE can run one step further ahead of
            # the exp drain, cutting the per-step lockstep stalls)
            if (mt == 4 and idx in (12, 14, 15, 16, 18, 19, 21, 22, 23)) \
                    or (mt == 6 and idx in (12, 14, 15, 16, 18, 19, 21,
                                            22, 23)):
                s = psP.tile([P, N], FP32, tag="p", name="s_extra")
            else:
                s = psS.tile([P, N], FP32, tag="s", name="s_att")
            for ch in range(2):
                nsl = slice(ch * 512, (ch + 1) * 512)
                nc.tensor.matmul(
                    s[:, nsl], kblk[:, p, h, mt, :], qdup[:, p, h, nsl],
                    start=True, stop=True)
            if mt in DVE_MTS:
                nc.vector.tensor_scalar(
                    a[:, mt, :].bitcast(I16), s[:],
                    sgn * EXP_A, EXP_B, MUL, ADD)
            else:
                nc.scalar.activation(a[:, mt, :], s[:], EXP,
                                     scale=sgn * EXPSC)
            if pend is not None:
                for nt in AV_SCHED.get(mt, ()):
                    emit_av_slice(pend, nt)
                if mt == 6:
                    emit_norm(pend)
                if mt == 7 and pend[1] == 1 and pend[2] == 1:
                    emit_transpose(pend[0])
            for fn in sched.get((idx, mt), ()):
                fn()
        return a

    # weight loads: wq then xqo on the scalar queue, wqo on sync (its
    # w16-ring slot frees only after the vmt fillers, so its DMA must not
    # block the gpsimd scatters or the first exps), the rest on gpsimd
    kmm = make_proj(d_wk, xqk, "wk16")
    qmm = make_proj(d_wq, xqk, "wq16", eng=nc.scalar)
    for t in range(3, CT):
        nc.scalar.dma_start(xv[:, t, :], d_v[t * P:(t + 1) * P, :])
    load_split(xqo, d_qo, nc.scalar)
    vmt = make_vproj()
    qomm = make_proj(d_wqo, xqo, "wqo16", eng=nc.sync)

    # prologue: zeros + co 0 and 4 of Q/K inline (the PE would otherwise
    # stall on input DMA here anyway), vsb tiles 0-3 before the first AV.
    # The attention-phase psum slots (psS "s", psO "o") are still free here,
    # so rotate the prologue projections through them — a single psP slot
    # would serialize each co_fn behind the previous one's DVE cast.
    def pro_ps():
        i = 0
        while True:
            yield psP.tile([P, N], FP32, tag="p", name="ps_pro")
            yield psS.tile([P, N], FP32, tag="s", name="ps_pro2")
            yield psO.tile([P, NT, P], FP32, tag="o",
                           name="ps_pro3").rearrange("p a b -> p (a b)")
            i += 1
    pro = pro_ps()
    for co in range(CT):
        zmemset(co)
    # q first (wq's ring is shorter, so it lands before wk finishes), and
    # half-granular sinks so the first S matmuls start ~3us earlier
    dup_sink_split(0, qmm(0, next(pro)))
    k_sink_split(0, kmm(0, next(pro)))
    nc.vector.memset(vsb[:, :, :, HD:HD + 1], 1.0)
    # co=4 before the vmt tiles: the vmt matmuls wait on the xv DMA anyway,
    # and this keeps the PE busy through that window
    k_sink(4, kmm(4, next(pro)))
    dup_sink(4, qmm(4, next(pro)))
    for mt in range(4):
        vmt(mt, next(pro))
    wp = wpool.tile([P, CT, C], BF16, tag="wf", name="wp", bufs=1)
    load_split(wp, d_wp, nc.gpsimd)

    heads = [(0, 0, 0), (0, 0, 1), (4, 0, 0), (4, 0, 1), (1, 0, 0),
             (1, 0, 1), (0, 1, 0), (0, 1, 1), (4, 1, 0), (4, 1, 1),
             (2, 0, 0), (2, 0, 1), (1, 1, 0), (1, 1, 1), (3, 0, 0),
             (3, 0, 1), (2, 1, 0), (2, 1, 1), (5, 0, 0), (5, 0, 1),
             (3, 1, 0), (3, 1, 1), (5, 1, 0), (5, 1, 1)]
    # Explicit filler schedule, (head idx, weave mt) -> work. Each projection
    # emits its matmuls early in a head (mt=2) and its sink (DVE cast +
    # gpsimd scatter) late (mt=7), so the cast is ready by the time the DVE
    # queue reaches it and never delays an exp. On transpose-heads (8, 10)
    # the slot must free before the mt=5 transpose: mm@1, sink@4.
    # qoco(p) overwrites qdup[:, p]: after head (p,0,1), before (p,1,0).
    pend_ps = {}

    def mm(key, fn, co):
        def run():
            pend_ps[key] = fn(co)
        return run

    def snk(key, fn, co):
        def run():
            fn(co, pend_ps.pop(key))
        return run

    sched = {
        (0, 1): [lambda: vmt(4)], (0, 3): [lambda: vmt(5)],
        (0, 5): [lambda: vmt(6)], (0, 7): [lambda: vmt(7)],
        (1, 2): [mm("q1", qmm, 1)], (1, 7): [snk("q1", dup_sink, 1)],
        (2, 2): [mm("k1", kmm, 1)], (2, 7): [snk("k1", k_sink, 1)],
        (3, 2): [mm("o0", qomm, 0)], (3, 7): [snk("o0", dup_sink, 0)],
        (4, 2): [mm("q2", qmm, 2)], (4, 7): [snk("q2", dup_sink, 2)],
        (5, 2): [mm("o4", qomm, 4)], (5, 7): [snk("o4", dup_sink, 4)],
        (6, 2): [mm("k2", kmm, 2)], (6, 7): [snk("k2", k_sink, 2)],
        (7, 2): [mm("q3", qmm, 3)], (7, 7): [snk("q3", dup_sink, 3)],
        (8, 1): [mm("o1", qomm, 1)], (8, 4): [snk("o1", dup_sink, 1)],
        (9, 2): [mm("k3", kmm, 3)], (9, 7): [snk("k3", k_sink, 3)],
        (10, 1): [mm("q5", qmm, 5)], (10, 4): [snk("q5", dup_sink, 5)],
        (11, 2): [mm("k5", kmm, 5)], (11, 7): [snk("k5", k_sink, 5)],
        (13, 2): [mm("o2", qomm, 2)], (13, 7): [snk("o2", dup_sink, 2)],
        (17, 2): [mm("o3", qomm, 3)], (17, 7): [snk("o3", dup_sink, 3)],
        (20, 2): [mm("o5", qomm, 5)], (20, 7): [snk("o5", dup_sink, 5)],
    }

    pend = None
    for idx, (p, att, h) in enumerate(heads):
        a = emit_head(idx, p, att, h, pend, sched)
        pend = (p, att, h, a)
    o = psO.tile([P, NT, P], FP32, tag="o", name="o_av")
    pend = pend + (o,)
    for nt in range(NT):
        emit_av_slice(pend, nt)
    emit_norm(pend)
    emit_transpose(pend[0])

    # ---- phase 3: output projection (alternating psum slots) ---------------
    def proj_nt(nt, ps):
        for base, wd in ((0, 512), (512, 256)):
            for ct in range(CT):
                nc.tensor.matmul(
                    ps[:, base:base + wd],
                    onormT[:, ct, nt * P:(nt + 1) * P],
                    wp[:, ct, base:base + wd],
                    start=(ct == 0), stop=(ct == CT - 1))
        osb = opool.tile([P, C], BF16, tag="out", name="osb")
        nc.vector.tensor_copy(osb[:], ps[:, 0:C])
        nc.sync.dma_start(d_out[nt * P:(nt + 1) * P, :], osb[:])

    for nt in range(NT):
        if nt % 2 == 0:
            ps = psP.tile([P, N], FP32, tag="p", name="ps_proj")
        else:
            # borrow the (now idle) o-slot: same 4KB, reshaped flat
            ps = psO.tile([P, NT, P], FP32, tag="o",
                          name="ps_proj2").rearrange("p a b -> p (a b)")
        proj_nt(nt, ps)

    ctx.close()


_NC = None


def _get_nc():
    global _NC
    if _NC is None:
        _NC = build_kernel()
    return _NC


def prepare_in_maps(x, y, w_qkv, w_proj, b_proj):
    x = np.asarray(x, np.float32)
    y = np.asarray(y, np.float32)
    w_qkv = np.asarray(w_qkv, np.float32)
    w_proj = np.asarray(w_proj, np.float32)

    bf = ml_dtypes.bfloat16
    tb = lambda a: np.ascontiguousarray(a.T).astype(bf)
    wqo16 = tb(w_qkv[0:C])
    wq16 = tb(w_qkv[C:2 * C])
    wk16 = tb(w_qkv[2 * C:3 * C])
    wv16 = tb(w_qkv[3 * C:4 * C])
    wp16 = tb(w_proj)
    eye16 = np.eye(P, dtype=bf)

    in_maps = []
    for i in range(8):
        b = i % 4
        isx = i < 4
        t_qk = x[b] if isx else y[b]
        t_qo = y[b] if isx else x[b]
        in_maps.append({
            "qk16": tb(t_qk), "qo16": tb(t_qo), "v16": tb(x[b]),
            "wq16": wq16, "wk16": wk16, "wqo16": wqo16, "wv16": wv16,
            "wp16": wp16, "eye16": eye16,
        })
    return in_maps


def kernel(x, y, w_qkv, w_proj, b_proj):
    nc = _get_nc()
    in_maps = prepare_in_maps(x, y, w_qkv, w_proj, b_proj)
    res = run_bass_kernel_spmd(nc, in_maps, list(range(8)))
    bpf = np.asarray(b_proj, np.float32)
    out_x = np.stack([np.asarray(res.results[b]["out"], np.float32)
                      for b in range(4)]) + bpf
    out_y = np.stack([np.asarray(res.results[4 + b]["out"], np.float32)
                      for b in range(4)]) + bpf
    return out_x.astype(np.float32), out_y.astype(np.float32)


if __name__ == "__main__":
    rng = np.random.default_rng(0)
    ins = {
        "x": rng.standard_normal((B, N, C), dtype=np.float32),
        "y": rng.standard_normal((B, N, C), dtype=np.float32),
        "w_qkv": (rng.standard_normal((4 * C, C)) * 0.02).astype(np.float32),
        "w_proj": (rng.standard_normal((C, C)) * 0.02).astype(np.float32),
        "b_proj": (rng.standard_normal(C) * 0.02).astype(np.float32),
    }
    ox, oy = kernel(**ins)
    print(ox.shape, oy.shape, ox.dtype)
